# revision 10
# baseline (speedup 1.0000x reference)
"""MoE (top-2 of 8 experts) Trainium2 kernel.

Strategy: expert-parallel across the 8 NeuronCores with 2-slot load
balancing. The router (a tiny [T,512]@[512,8] matmul + softmax + top-k,
~0.02% of the layer's FLOPs) runs on host bit-identically to the
reference (jax on CPU). Tokens are gathered per expert on host and
assigned to per-core (segment A, segment B) slots of fixed sizes
(C1, C2): the heaviest expert is split across two cores' A slots, the
lightest across those cores' B slots, and each remaining expert fills
one core's A+B. This balances per-core work to
max(2nd-heaviest, heaviest/2 + lightest/2) tokens instead of padding
every core to the heaviest expert.

Each segment computes its expert's FFN in a transposed layout (features
on partitions, tokens on the moving/free axis):

    outT = (w2.T @ gelu(w1.T @ xT + b1) + b2) * gate

so both matmuls chain on the TensorEngine with no transposes, and the
b1/b2 biases are free per-partition operands. The gate multiply uses a
partition-broadcast gate row. Host scatter-adds the two expert
contributions per token back into the full [B,S,D] output.

Matmul operands are bf16 (same 1 col/cycle PE rate as fp32r, half the
HBM traffic, FWL weight loads; ~4e-3 rel err end to end, vs the 2e-2
budget). PSUM accumulation stays fp32. ~30 dummy matmuls on a memset
scratch tile run while the first input DMAs stream, lifting the PE HAM
clock gate from 1.2 to 2.4 GHz before the real matmul stream begins.

All device inputs are packed on host into contiguous blocks laid out in
exactly the order the kernel consumes them: HWDGE drains the sync ring
FIFO, so consumption-ordered contiguous blocks give both full DMA
bandwidth and earliest possible compute start. Segment B's weights sit
behind segment A's in the ring and stream during segment A's compute.
"""

import os
import sys

sys.path.insert(0, "/opt/trn_rl_repo")

import ml_dtypes
import numpy as np

TOP_K = 2
N_CORES = 8
P = 128  # SBUF partitions

# Matmul dtype: "bfloat16" (1 cyc/row, FWL weight loads, half the DMA
# bytes, ~4e-3 rel err), "float32r" (1 cyc/row at N>=256, ~2e-4 rel err)
# or "float32" (exact, 4 cyc/row).
MM_DT = os.environ.get("MOE_MM_DT", "bfloat16")
NTILE = 512  # moving-operand (token) tile; PSUM-bank cap for f32 accum
ACT_FUNC = os.environ.get("MOE_ACT_FUNC", "Gelu")  # CoreSim lacks Gelu; Tanh for sim
N_WARM = 30  # dummy matmuls to lift the PE HAM clock-gate during input DMA

_NP_MM_DT = {
    "bfloat16": ml_dtypes.bfloat16,
    "float32r": np.float32,
    "float32": np.float32,
}


def _route(x_flat, gate_w, gate_b):
    """Reference router, bit-identical: jax on CPU."""
    import jax
    import jax.numpy as jnp

    with jax.default_device(jax.devices("cpu")[0]):
        logits = jnp.asarray(x_flat) @ jnp.asarray(gate_w) + jnp.asarray(gate_b)
        raw_weights = jax.nn.softmax(logits, axis=-1)
        top_w, top_idx = jax.lax.top_k(raw_weights, TOP_K)
        return np.asarray(top_w), np.asarray(top_idx)


def _tile_sizes(C):
    return [min(NTILE, C - c0) for c0 in range(0, C, NTILE)]


def _pack_xt(XT, C, D):
    """[D, C] -> consumption-ordered per-tile [P][KT][csz] blocks."""
    KT = D // P
    blocks = []
    for i, csz in enumerate(_tile_sizes(C)):
        c0 = i * NTILE
        blocks.append(
            XT.reshape(KT, P, C)[:, :, c0 : c0 + csz].transpose(1, 0, 2).ravel()
        )
    return np.concatenate(blocks)


def _pack_weights(w1e, b1e, w2e, b2e, D, H):
    """One expert's weights in the kernel's blocked layouts."""
    np_mm = _NP_MM_DT[MM_DT]
    KT, MT, DT = D // P, H // P, D // P
    MTG = MT // 4
    return (
        np.ascontiguousarray(
            w1e.reshape(KT, P, MT, P).transpose(2, 1, 0, 3).astype(np_mm)
        ),
        np.ascontiguousarray(b1e.reshape(MT, P).T),
        np.ascontiguousarray(
            w2e.reshape(MTG, 4, P, D).transpose(0, 2, 1, 3).astype(np_mm)
        ),
        np.ascontiguousarray(b2e.reshape(DT, P).T),
    )


def _unpack_out(flat, tiles, D):
    """Per-tile [P][DT*csz] blocks -> outT [D, sum(tiles)]."""
    DT = D // P
    flat = np.asarray(flat).astype(np.float32)
    outT = np.empty((D, sum(tiles)), np.float32)
    off = 0
    c0 = 0
    for csz in tiles:
        blk = flat[off : off + P * DT * csz].reshape(P, DT, csz)
        outT[:, c0 : c0 + csz] = blk.transpose(1, 0, 2).reshape(D, csz)
        off += P * DT * csz
        c0 += csz
    return outT


def _build_program(C1, C2, D, H, mm_dt_name):
    """Build the per-core Bass program (identical on all cores).

    Two sequential segments (sizes C1 then C2), each with its own
    expert weight set; token/gate/output buffers are concatenated.
    """
    import concourse.bass as bass
    import concourse.mybir as mybir
    import concourse.tile as tile
    from concourse import bacc
    from concourse.tile_rust import add_dep_helper

    f32 = mybir.dt.float32
    mm_dt = getattr(mybir.dt, mm_dt_name)
    act = getattr(mybir.ActivationFunctionType, ACT_FUNC)
    KT = D // P  # 4  k-tiles for matmul1 (contraction over D)
    MT = H // P  # 16 m-tiles (H rows of hT)
    DT = D // P  # 4  d-tiles of the output
    MTG = MT // 4  # 4  w2 row-block groups
    C = C1 + C2

    nc = bacc.Bacc(None, target_bir_lowering=False, debug=False)
    xt_h = nc.dram_tensor("xt", [P * KT * C], mm_dt, kind="ExternalInput")
    g_h = nc.dram_tensor("g", [1, C], f32, kind="ExternalInput")
    w_h = {}
    for s in ("a", "b"):
        w_h[s] = (
            nc.dram_tensor(f"w1{s}", [MT, P, KT, P], mm_dt, kind="ExternalInput"),
            nc.dram_tensor(f"b1{s}", [P, MT], f32, kind="ExternalInput"),
            nc.dram_tensor(f"w2{s}", [MTG, P, 4, D], mm_dt, kind="ExternalInput"),
            nc.dram_tensor(f"b2{s}", [P, DT], f32, kind="ExternalInput"),
        )
    out_h = nc.dram_tensor("out", [P * DT * C], mm_dt, kind="ExternalOutput")

    with tile.TileContext(nc) as tc:
        with (
            tc.tile_pool(name="weights", bufs=1) as wpool,
            tc.tile_pool(name="xio", bufs=2) as xio,
            tc.tile_pool(name="gio", bufs=2) as gio,
            tc.tile_pool(name="oio", bufs=3) as oio,
            tc.tile_pool(name="hbuf", bufs=1) as hbuf,
            tc.tile_pool(name="ps1", bufs=4, space=bass.MemorySpace.PSUM) as ps1,
            # matmul2 keeps DT banks live across its whole m-loop; bufs=1
            # per d-tag (release happens at the DVE evacuation, early in
            # the next n-tile's matmul1 phase). 4 + 4 = 8 banks.
            tc.tile_pool(name="ps2", bufs=1, space=bass.MemorySpace.PSUM) as ps2,
        ):
            # DMA issue order == consumption order (sync ring is FIFO):
            # xt[a,0], segment-A weights, gate row, then segment-B
            # weights, then per-tile IO. bf16 streams fast enough that no
            # priority gating is needed: every upfront block lands well
            # before its first consumer.
            def load_weights(s):
                w1h, b1h, w2h, b2h = w_h[s]
                w1_t = [
                    wpool.tile([P, KT, P], mm_dt, name=f"w1{s}_{m}") for m in range(MT)
                ]
                for m in range(MT):
                    nc.sync.dma_start(out=w1_t[m], in_=w1h.ap()[m])
                b1_sb = wpool.tile([P, MT], f32, name=f"b1{s}")
                nc.sync.dma_start(out=b1_sb, in_=b1h.ap())
                b2_sb = wpool.tile([P, DT], f32, name=f"b2{s}")
                nc.sync.dma_start(out=b2_sb, in_=b2h.ap())
                w2_t = []
                for mtg in range(MTG):
                    t = wpool.tile([P, 4, D], mm_dt, name=f"w2{s}_{mtg}")
                    nc.sync.dma_start(out=t, in_=w2h.ap()[mtg])
                    w2_t.append(t)
                return w1_t, b1_sb, w2_t, b2_sb

            sizes_a, sizes_b = _tile_sizes(C1), _tile_sizes(C2)

            # first xt tile, then segment A weights
            xt0 = xio.tile([P, KT, sizes_a[0]], mm_dt, tag="xt", name="xt0")
            nc.sync.dma_start(
                out=xt0,
                in_=xt_h.ap()[0 : P * KT * sizes_a[0]].rearrange(
                    "(p kt c) -> p kt c", p=P, kt=KT
                ),
            )
            wts_a = load_weights("a")
            # broadcast the gate row across partitions in one HWDGE DMA
            g_full = gio.tile([P, C], f32, name="g_full")
            nc.sync.dma_start(out=g_full, in_=g_h.ap().partition_broadcast(P))
            wts_b = load_weights("b")

            # PE warmup: the HAM clock gate holds the PE at 1.2 GHz until
            # ~3.4us of sustained matmul activity. Real matmuls can't
            # start until xt0+w1a_0 land (~9us in); dummy matmuls on a
            # memset scratch tile need no DMA, so they run from t~7.5us
            # and lift the clock toward 2.4 GHz before the real stream
            # begins. They borrow ps2_0's PSUM bank (released long before
            # matmul2 needs it).
            warm_sb = wpool.tile([P, P], mm_dt, name="warm")
            nc.gpsimd.memset(warm_sb[:, :], 0.0)
            warm_ps = ps2.tile([P, NTILE], f32, tag="ps2_0", name="ps2_0")
            for _ in range(N_WARM):
                nc.tensor.matmul(
                    warm_ps[:, 0:P], lhsT=warm_sb, rhs=warm_sb, start=True, stop=True
                )

            state = {"xt_off": 0, "out_off": 0, "c0": 0, "prev_first_act": None}

            def run_segment(sizes, wts, xt_first=None):
                w1_t, b1_sb, w2_t, b2_sb = wts
                for n, csz in enumerate(sizes):
                    if n == 0 and xt_first is not None:
                        xt_t = xt_first
                    else:
                        xt_t = xio.tile([P, KT, csz], mm_dt, tag="xt", name="xt")
                        dma = nc.sync.dma_start(
                            out=xt_t,
                            in_=xt_h.ap()[
                                state["xt_off"] : state["xt_off"] + P * KT * csz
                            ].rearrange("(p kt c) -> p kt c", p=P, kt=KT),
                        )
                        if state["prev_first_act"] is not None:
                            add_dep_helper(
                                dma.ins,
                                state["prev_first_act"].ins,
                                reason="stagger xt load",
                            )
                    state["xt_off"] += P * KT * csz
                    g_t = g_full[:, state["c0"] : state["c0"] + csz]
                    state["c0"] += csz
                    hT = hbuf.tile([P, MT, csz], mm_dt, tag="hT", name="hT")
                    first_act = None
                    for m in range(MT):
                        pst = ps1.tile([P, csz], f32, tag="ps1", name="ps1")
                        for kt in range(KT):
                            nc.tensor.matmul(
                                pst,
                                lhsT=w1_t[m][:, kt, :],
                                rhs=xt_t[:, kt, :],
                                start=(kt == 0),
                                stop=(kt == KT - 1),
                            )
                        a = nc.scalar.activation(
                            out=hT[:, m, :],
                            in_=pst,
                            func=act,
                            bias=b1_sb[:, m : m + 1],
                            scale=1.0,
                        )
                        if m == 0:
                            first_act = a
                    state["prev_first_act"] = first_act
                    # matmul2 with m as the OUTER loop: w2 blocks are
                    # consumed in DMA-arrival order. Needs DT live PSUM
                    # banks.
                    pso = [
                        ps2.tile([P, csz], f32, tag=f"ps2_{d}", name=f"ps2_{d}")
                        for d in range(DT)
                    ]
                    for m in range(MT):
                        for d in range(DT):
                            nc.tensor.matmul(
                                pso[d],
                                lhsT=w2_t[m // 4][:, m % 4, d * P : (d + 1) * P],
                                rhs=hT[:, m, :],
                                start=(m == 0),
                                stop=(m == MT - 1),
                            )
                    ot = oio.tile([P, DT, csz], mm_dt, tag="ot", name="ot")
                    for d in range(DT):
                        nc.vector.scalar_tensor_tensor(
                            out=ot[:, d, :],
                            in0=pso[d],
                            scalar=b2_sb[:, d : d + 1],
                            in1=g_t,
                            op0=mybir.AluOpType.add,
                            op1=mybir.AluOpType.mult,
                        )
                    # one store per tile; [P][DT*csz] keeps the DMA
                    # descriptor 2D-contiguous (4KB partition lines)
                    nc.sync.dma_start(
                        out=out_h.ap()[
                            state["out_off"] : state["out_off"] + P * DT * csz
                        ].rearrange("(p x) -> p x", p=P),
                        in_=ot[:, :, :],
                    )
                    state["out_off"] += P * DT * csz

            run_segment(sizes_a, wts_a, xt_first=xt0)
            run_segment(sizes_b, wts_b)

    nc.compile()
    return nc


def _run(nc, in_maps, trace=False):
    from concourse.bass_utils import run_bass_kernel_spmd

    if trace:
        # register the NTFF profiling hook (missing antenv.axon_hooks shim)
        import types

        import antenv

        if not hasattr(antenv, "axon_hooks"):
            mod = types.ModuleType("antenv.axon_hooks")
            _hook = [None]
            mod.set_axon_ntff_profile_hook = lambda h: _hook.__setitem__(0, h)
            mod.get_axon_ntff_profile_hook = lambda: _hook[0]
            sys.modules["antenv.axon_hooks"] = mod
            antenv.axon_hooks = mod
            from trn_agent_boot.trn_boot import _ntff_profile_via_ctypes

            mod.set_axon_ntff_profile_hook(
                _ntff_profile_via_ctypes("/opt/axon/libaxon_pjrt.so")
            )
    return run_bass_kernel_spmd(
        nc, in_maps, core_ids=list(range(N_CORES)), trace=trace
    )


def _round8(v):
    return ((int(v) + 7) // 8) * 8


def _assign_slots(counts):
    """2-slot balanced assignment.

    Returns (C1, C2, cores) where cores[i] = [(expert, lo, hi), ...] of
    length 2: core i computes tokens [lo:hi) of each expert in its A/B
    slots. The heaviest expert splits across cores 0,1's A slots; the
    lightest across their B slots; each middle expert fills one core.
    """
    order = sorted(range(len(counts)), key=lambda e: -counts[e])
    top, bot, mids = order[0], order[-1], order[1:-1]
    C1 = _round8(max((counts[top] + 1) // 2, NTILE))
    second = counts[order[1]] if len(order) > 1 else 0
    C2 = _round8(max(second - C1, (counts[bot] + 1) // 2, NTILE))
    cores = []
    th, bh = (counts[top] + 1) // 2, (counts[bot] + 1) // 2
    cores.append([(top, 0, th), (bot, 0, bh)])
    cores.append([(top, th, counts[top]), (bot, bh, counts[bot])])
    for e in mids:
        cut = min(C1, counts[e])
        cores.append([(e, 0, cut), (e, cut, counts[e])])
    return C1, C2, cores


def kernel(x, gate_w, gate_b, w1, b1, w2, b2, _trace=False):
    x = np.ascontiguousarray(np.asarray(x, dtype=np.float32))
    gate_w = np.asarray(gate_w, dtype=np.float32)
    gate_b = np.asarray(gate_b, dtype=np.float32)
    w1 = np.asarray(w1, dtype=np.float32)
    b1 = np.asarray(b1, dtype=np.float32)
    w2 = np.asarray(w2, dtype=np.float32)
    b2 = np.asarray(b2, dtype=np.float32)

    B, S, D = x.shape
    E = gate_w.shape[1]
    H = w1.shape[2]
    assert E == N_CORES
    T = B * S
    x_flat = x.reshape(T, D)

    top_w, top_idx = _route(x_flat, gate_w, gate_b)

    toks, gvals = [], []
    for e in range(E):
        mask = top_idx == e  # [T, K]; at most one True per row
        t_ids = np.nonzero(mask.any(axis=1))[0]
        toks.append(t_ids)
        gvals.append(top_w[mask].astype(np.float32))
    counts = [len(t) for t in toks]
    C1, C2, cores = _assign_slots(counts)
    caps = (C1, C2)
    np_mm = _NP_MM_DT[MM_DT]

    in_maps = []
    for core in cores:
        xts, gs = [], []
        m = {}
        for s, (cap, (e, lo, hi)) in enumerate(zip(caps, core)):
            cnt = hi - lo
            XT = np.zeros((D, cap), np.float32)
            XT[:, :cnt] = x_flat[toks[e][lo:hi]].T
            G = np.zeros(cap, np.float32)
            G[:cnt] = gvals[e][lo:hi]
            xts.append(_pack_xt(XT, cap, D))
            gs.append(G)
            sn = "ab"[s]
            w1p, b1p, w2p, b2p = _pack_weights(w1[e], b1[e], w2[e], b2[e], D, H)
            m[f"w1{sn}"], m[f"b1{sn}"] = w1p, b1p
            m[f"w2{sn}"], m[f"b2{sn}"] = w2p, b2p
        m["xt"] = np.ascontiguousarray(np.concatenate(xts).astype(np_mm))
        m["g"] = np.ascontiguousarray(np.concatenate(gs).reshape(1, C1 + C2))
        in_maps.append(m)

    nc = _build_program(C1, C2, D, H, MM_DT)
    res = _run(nc, in_maps, trace=_trace)
    global _LAST_RES
    _LAST_RES = res

    out_flat = np.zeros((T, D), np.float32)
    tiles = _tile_sizes(C1) + _tile_sizes(C2)
    for i, core in enumerate(cores):
        outT = _unpack_out(res.results[i]["out"], tiles, D)
        for s, (cap, (e, lo, hi)) in enumerate(zip(caps, core)):
            cnt = hi - lo
            c0 = s * C1
            out_flat[toks[e][lo:hi]] += outT[:, c0 : c0 + cnt].T

    out = out_flat.reshape(B, S, D)
    if _trace:
        return out, res.exec_time_ns
    return out


# revision 14
# speedup vs baseline: 1.0526x; 1.0526x over previous
"""MoE (top-2 of 8 experts) Trainium2 kernel.

Strategy: expert-parallel across the 8 NeuronCores with 2-slot load
balancing. The router (a tiny [T,512]@[512,8] matmul + softmax + top-k,
~0.02% of the layer's FLOPs) runs on host bit-identically to the
reference (jax on CPU). Tokens are gathered per expert on host and
assigned to per-core (segment A, segment B) slots of fixed sizes
(C1, C2): the heaviest expert is split across two cores' A slots, the
lightest across those cores' B slots, and each remaining expert fills
one core's A+B. This balances per-core work to
max(2nd-heaviest, heaviest/2 + lightest/2) tokens instead of padding
every core to the heaviest expert.

Each segment computes its expert's FFN in a transposed layout (features
on partitions, tokens on the moving/free axis):

    outT = (w2.T @ gelu(w1.T @ xT + b1) + b2) * gate

so both matmuls chain on the TensorEngine with no transposes, and the
b1/b2 biases are free per-partition operands. The gate multiply uses a
partition-broadcast gate row. Host scatter-adds the two expert
contributions per token back into the full [B,S,D] output.

Matmul operands are bf16 (same 1 col/cycle PE rate as fp32r, half the
HBM traffic, FWL weight loads; ~4e-3 rel err end to end, vs the 2e-2
budget). PSUM accumulation stays fp32. ~30 dummy matmuls on a memset
scratch tile run while the first input DMAs stream, lifting the PE HAM
clock gate from 1.2 to 2.4 GHz before the real matmul stream begins.

All device inputs are packed on host into contiguous blocks laid out in
exactly the order the kernel consumes them: HWDGE drains the sync ring
FIFO, so consumption-ordered contiguous blocks give both full DMA
bandwidth and earliest possible compute start. Segment B's weights sit
behind segment A's in the ring and stream during segment A's compute.
"""

import os
import sys

sys.path.insert(0, "/opt/trn_rl_repo")

import ml_dtypes
import numpy as np

TOP_K = 2
N_CORES = 8
P = 128  # SBUF partitions

# Matmul dtype: "bfloat16" (1 cyc/row, FWL weight loads, half the DMA
# bytes, ~4e-3 rel err), "float32r" (1 cyc/row at N>=256, ~2e-4 rel err)
# or "float32" (exact, 4 cyc/row).
MM_DT = os.environ.get("MOE_MM_DT", "bfloat16")
NTILE = 512  # moving-operand (token) tile; PSUM-bank cap for f32 accum
ACT_FUNC = os.environ.get("MOE_ACT_FUNC", "Gelu")  # CoreSim lacks Gelu; Tanh for sim
N_WARM = 30  # dummy matmuls to lift the PE HAM clock-gate during input DMA

_NP_MM_DT = {
    "bfloat16": ml_dtypes.bfloat16,
    "float32r": np.float32,
    "float32": np.float32,
}


def _route(x_flat, gate_w, gate_b):
    """Reference router, bit-identical: jax on CPU."""
    import jax
    import jax.numpy as jnp

    with jax.default_device(jax.devices("cpu")[0]):
        logits = jnp.asarray(x_flat) @ jnp.asarray(gate_w) + jnp.asarray(gate_b)
        raw_weights = jax.nn.softmax(logits, axis=-1)
        top_w, top_idx = jax.lax.top_k(raw_weights, TOP_K)
        return np.asarray(top_w), np.asarray(top_idx)


def _tile_sizes(C):
    return [min(NTILE, C - c0) for c0 in range(0, C, NTILE)]


def _pack_xt(XT, C, D):
    """[D, C] -> consumption-ordered per-tile [P][KT][csz] blocks."""
    KT = D // P
    blocks = []
    for i, csz in enumerate(_tile_sizes(C)):
        c0 = i * NTILE
        blocks.append(
            XT.reshape(KT, P, C)[:, :, c0 : c0 + csz].transpose(1, 0, 2).ravel()
        )
    return np.concatenate(blocks)


def _pack_weights(w1e, b1e, w2e, b2e, D, H):
    """One expert's weights in the kernel's blocked layouts."""
    np_mm = _NP_MM_DT[MM_DT]
    KT, MT, DT = D // P, H // P, D // P
    MTG = MT // 4
    return (
        np.ascontiguousarray(
            w1e.reshape(KT, P, MT, P).transpose(2, 1, 0, 3).astype(np_mm)
        ),
        np.ascontiguousarray(b1e.reshape(MT, P).T),
        np.ascontiguousarray(
            w2e.reshape(MTG, 4, P, D).transpose(0, 2, 1, 3).astype(np_mm)
        ),
        np.ascontiguousarray(b2e.reshape(DT, P).T),
    )


def _unpack_out(flat, tiles, D):
    """Per-tile [P][DT*csz] blocks -> outT [D, sum(tiles)]."""
    DT = D // P
    flat = np.asarray(flat).astype(np.float32)
    outT = np.empty((D, sum(tiles)), np.float32)
    off = 0
    c0 = 0
    for csz in tiles:
        blk = flat[off : off + P * DT * csz].reshape(P, DT, csz)
        outT[:, c0 : c0 + csz] = blk.transpose(1, 0, 2).reshape(D, csz)
        off += P * DT * csz
        c0 += csz
    return outT


def _build_program(C1, C2, D, H, mm_dt_name):
    """Build the per-core Bass program (identical on all cores).

    Two sequential segments (sizes C1 then C2), each with its own
    expert weight set; token/gate/output buffers are concatenated.
    """
    import concourse.bass as bass
    import concourse.mybir as mybir
    import concourse.tile as tile
    from concourse import bacc
    from concourse.tile_rust import add_dep_helper

    f32 = mybir.dt.float32
    mm_dt = getattr(mybir.dt, mm_dt_name)
    act = getattr(mybir.ActivationFunctionType, ACT_FUNC)
    KT = D // P  # 4  k-tiles for matmul1 (contraction over D)
    MT = H // P  # 16 m-tiles (H rows of hT)
    DT = D // P  # 4  d-tiles of the output
    MTG = MT // 4  # 4  w2 row-block groups
    C = C1 + C2

    nc = bacc.Bacc(None, target_bir_lowering=False, debug=False)
    xt_h = nc.dram_tensor("xt", [P * KT * C], mm_dt, kind="ExternalInput")
    g_h = nc.dram_tensor("g", [1, C], f32, kind="ExternalInput")
    w_h = {}
    for s in ("a", "b"):
        w_h[s] = (
            nc.dram_tensor(f"w1{s}", [MT, P, KT, P], mm_dt, kind="ExternalInput"),
            nc.dram_tensor(f"b1{s}", [P, MT], f32, kind="ExternalInput"),
            nc.dram_tensor(f"w2{s}", [MTG, P, 4, D], mm_dt, kind="ExternalInput"),
            nc.dram_tensor(f"b2{s}", [P, DT], f32, kind="ExternalInput"),
        )
    out_h = nc.dram_tensor("out", [P * DT * C], mm_dt, kind="ExternalOutput")

    with tile.TileContext(nc) as tc:
        with (
            tc.tile_pool(name="weights", bufs=1) as wpool,
            tc.tile_pool(name="xio", bufs=2) as xio,
            tc.tile_pool(name="gio", bufs=2) as gio,
            tc.tile_pool(name="oio", bufs=3) as oio,
            tc.tile_pool(name="hbuf", bufs=1) as hbuf,
            tc.tile_pool(name="ps1", bufs=4, space=bass.MemorySpace.PSUM) as ps1,
            # matmul2 keeps DT banks live across its whole m-loop; bufs=1
            # per d-tag (release happens at the DVE evacuation, early in
            # the next n-tile's matmul1 phase). 4 + 4 = 8 banks.
            tc.tile_pool(name="ps2", bufs=1, space=bass.MemorySpace.PSUM) as ps2,
        ):
            # DMA issue order == consumption order, and DMAs spread
            # round-robin over the hardware rings and run CONCURRENTLY,
            # so everything not needed early must be held back behind a
            # compute gate or it steals HBM bandwidth from the critical
            # xt0+w1a stream (measured: ungating all weights starves
            # matmul1 of tile A0 for ~8us and re-throttles the HAM).
            # Stream plan: xt0, w1a_0..3, b1a land immediately; w1a_4..15
            # just-in-time behind matmul gates; b2a/w2a/g behind the
            # first tile's early gelu; segment-B weights are emitted
            # inside the loop after tile A1's xt load, gated on tile A1's
            # first activation (~1/3 into segment A) — a quiet window
            # long before segment B's compute begins.
            sizes_a, sizes_b = _tile_sizes(C1), _tile_sizes(C2)

            xt0 = xio.tile([P, KT, sizes_a[0]], mm_dt, tag="xt", name="xt0")
            nc.sync.dma_start(
                out=xt0,
                in_=xt_h.ap()[0 : P * KT * sizes_a[0]].rearrange(
                    "(p kt c) -> p kt c", p=P, kt=KT
                ),
            )
            # segment A weights: w1a 0..3 ungated, b1a, w1a 4..15
            w1h_a, b1h_a, w2h_a, b2h_a = w_h["a"]
            w1a_t = [
                wpool.tile([P, KT, P], mm_dt, name=f"w1a_{m}") for m in range(MT)
            ]
            w1a_dmas = [
                nc.sync.dma_start(out=w1a_t[m], in_=w1h_a.ap()[m]) for m in range(4)
            ]
            b1a_sb = wpool.tile([P, MT], f32, name="b1a")
            nc.sync.dma_start(out=b1a_sb, in_=b1h_a.ap())
            for m in range(4, MT):
                w1a_dmas.append(nc.sync.dma_start(out=w1a_t[m], in_=w1h_a.ap()[m]))
            b2a_sb = wpool.tile([P, DT], f32, name="b2a")
            late_a_dmas = [nc.sync.dma_start(out=b2a_sb, in_=b2h_a.ap())]
            # broadcast the gate row across partitions in one HWDGE DMA
            g_full = gio.tile([P, C], f32, name="g_full")
            late_a_dmas.append(
                nc.sync.dma_start(out=g_full, in_=g_h.ap().partition_broadcast(P))
            )
            w2a_t = []
            for mtg in range(MTG):
                t = wpool.tile([P, 4, D], mm_dt, name=f"w2a_{mtg}")
                late_a_dmas.append(nc.sync.dma_start(out=t, in_=w2h_a.ap()[mtg]))
                w2a_t.append(t)
            wts_a = (w1a_t, b1a_sb, w2a_t, b2a_sb)

            def emit_seg_b_weights():
                w1h, b1h, w2h, b2h = w_h["b"]
                w1_t = [
                    wpool.tile([P, KT, P], mm_dt, name=f"w1b_{m}") for m in range(MT)
                ]
                dmas = [
                    nc.sync.dma_start(out=w1_t[m], in_=w1h.ap()[m]) for m in range(MT)
                ]
                b1_sb = wpool.tile([P, MT], f32, name="b1b")
                dmas.append(nc.sync.dma_start(out=b1_sb, in_=b1h.ap()))
                b2_sb = wpool.tile([P, DT], f32, name="b2b")
                dmas.append(nc.sync.dma_start(out=b2_sb, in_=b2h.ap()))
                w2_t = []
                for mtg in range(MTG):
                    t = wpool.tile([P, 4, D], mm_dt, name=f"w2b_{mtg}")
                    dmas.append(nc.sync.dma_start(out=t, in_=w2h.ap()[mtg]))
                    w2_t.append(t)
                return (w1_t, b1_sb, w2_t, b2_sb), dmas

            # PE warmup: the HAM clock gate holds the PE at 1.2 GHz until
            # ~3.4us of sustained matmul activity. Real matmuls can't
            # start until xt0+w1a_0 land (~9us in); dummy matmuls on a
            # memset scratch tile need no DMA, so they run from t~7.5us
            # and lift the clock toward 2.4 GHz before the real stream
            # begins. They borrow ps2_0's PSUM bank (released long before
            # matmul2 needs it).
            warm_sb = wpool.tile([P, P], mm_dt, name="warm")
            nc.gpsimd.memset(warm_sb[:, :], 0.0)
            warm_ps = ps2.tile([P, NTILE], f32, tag="ps2_0", name="ps2_0")
            for _ in range(N_WARM):
                nc.tensor.matmul(
                    warm_ps[:, 0:P], lhsT=warm_sb, rhs=warm_sb, start=True, stop=True
                )

            state = {
                "xt_off": 0,
                "out_off": 0,
                "c0": 0,
                "prev_first_act": None,
                "wts_b": None,
            }

            def run_segment(seg, sizes, wts, xt_first=None):
                w1_t, b1_sb, w2_t, b2_sb = wts
                for n, csz in enumerate(sizes):
                    if n == 0 and xt_first is not None:
                        xt_t = xt_first
                    else:
                        xt_t = xio.tile([P, KT, csz], mm_dt, tag="xt", name="xt")
                        dma = nc.sync.dma_start(
                            out=xt_t,
                            in_=xt_h.ap()[
                                state["xt_off"] : state["xt_off"] + P * KT * csz
                            ].rearrange("(p kt c) -> p kt c", p=P, kt=KT),
                        )
                        if state["prev_first_act"] is not None:
                            add_dep_helper(
                                dma.ins,
                                state["prev_first_act"].ins,
                                reason="stagger xt load",
                            )
                    state["xt_off"] += P * KT * csz
                    # segment-B weights enter the ring right after tile
                    # A1's xt load; their gate (act A1 m0) is attached
                    # below once that activation exists
                    pending_b = None
                    if seg == "a" and n == min(1, len(sizes) - 1):
                        state["wts_b"], pending_b = emit_seg_b_weights()
                    g_t = g_full[:, state["c0"] : state["c0"] + csz]
                    state["c0"] += csz
                    hT = hbuf.tile([P, MT, csz], mm_dt, tag="hT", name="hT")
                    first_act = None
                    for m in range(MT):
                        pst = ps1.tile([P, csz], f32, tag="ps1", name="ps1")
                        for kt in range(KT):
                            mm = nc.tensor.matmul(
                                pst,
                                lhsT=w1_t[m][:, kt, :],
                                rhs=xt_t[:, kt, :],
                                start=(kt == 0),
                                stop=(kt == KT - 1),
                            )
                            # just-in-time w1a streaming with one group
                            # of slack: blocks m+4..m+7 released by the
                            # first matmul that consumes block m
                            if seg == "a" and n == 0 and kt == 0 and m % 4 == 0:
                                for j in range(m + 4, min(m + 8, MT)):
                                    add_dep_helper(
                                        w1a_dmas[j].ins,
                                        mm.ins,
                                        reason="stagger w1 load",
                                    )
                        a = nc.scalar.activation(
                            out=hT[:, m, :],
                            in_=pst,
                            func=act,
                            bias=b1_sb[:, m : m + 1],
                            scale=1.0,
                        )
                        if m == 0:
                            first_act = a
                            if pending_b is not None:
                                for dma in pending_b:
                                    add_dep_helper(
                                        dma.ins, a.ins, reason="stagger segB weights"
                                    )
                        if seg == "a" and n == 0 and m == 2:
                            for dma in late_a_dmas:
                                add_dep_helper(
                                    dma.ins, a.ins, reason="stagger w2a/g load"
                                )
                    state["prev_first_act"] = first_act
                    # matmul2 with m as the OUTER loop: w2 blocks are
                    # consumed in DMA-arrival order. Needs DT live PSUM
                    # banks.
                    pso = [
                        ps2.tile([P, csz], f32, tag=f"ps2_{d}", name=f"ps2_{d}")
                        for d in range(DT)
                    ]
                    for m in range(MT):
                        for d in range(DT):
                            nc.tensor.matmul(
                                pso[d],
                                lhsT=w2_t[m // 4][:, m % 4, d * P : (d + 1) * P],
                                rhs=hT[:, m, :],
                                start=(m == 0),
                                stop=(m == MT - 1),
                            )
                    ot = oio.tile([P, DT, csz], mm_dt, tag="ot", name="ot")
                    for d in range(DT):
                        nc.vector.scalar_tensor_tensor(
                            out=ot[:, d, :],
                            in0=pso[d],
                            scalar=b2_sb[:, d : d + 1],
                            in1=g_t,
                            op0=mybir.AluOpType.add,
                            op1=mybir.AluOpType.mult,
                        )
                    # one store per tile; [P][DT*csz] keeps the DMA
                    # descriptor 2D-contiguous (4KB partition lines)
                    nc.sync.dma_start(
                        out=out_h.ap()[
                            state["out_off"] : state["out_off"] + P * DT * csz
                        ].rearrange("(p x) -> p x", p=P),
                        in_=ot[:, :, :],
                    )
                    state["out_off"] += P * DT * csz

            run_segment("a", sizes_a, wts_a, xt_first=xt0)
            run_segment("b", sizes_b, state["wts_b"])

    nc.compile()
    return nc


def _run(nc, in_maps, trace=False):
    from concourse.bass_utils import run_bass_kernel_spmd

    if trace:
        # register the NTFF profiling hook (missing antenv.axon_hooks shim)
        import types

        import antenv

        if not hasattr(antenv, "axon_hooks"):
            mod = types.ModuleType("antenv.axon_hooks")
            _hook = [None]
            mod.set_axon_ntff_profile_hook = lambda h: _hook.__setitem__(0, h)
            mod.get_axon_ntff_profile_hook = lambda: _hook[0]
            sys.modules["antenv.axon_hooks"] = mod
            antenv.axon_hooks = mod
            from trn_agent_boot.trn_boot import _ntff_profile_via_ctypes

            mod.set_axon_ntff_profile_hook(
                _ntff_profile_via_ctypes("/opt/axon/libaxon_pjrt.so")
            )
    return run_bass_kernel_spmd(
        nc, in_maps, core_ids=list(range(N_CORES)), trace=trace
    )


def _round8(v):
    return ((int(v) + 7) // 8) * 8


def _assign_slots(counts):
    """2-slot balanced assignment.

    Returns (C1, C2, cores) where cores[i] = [(expert, lo, hi), ...] of
    length 2: core i computes tokens [lo:hi) of each expert in its A/B
    slots. The heaviest expert splits across cores 0,1's A slots; the
    lightest across their B slots; each middle expert fills one core.
    """
    order = sorted(range(len(counts)), key=lambda e: -counts[e])
    top, bot, mids = order[0], order[-1], order[1:-1]
    C1 = _round8(max((counts[top] + 1) // 2, NTILE))
    second = counts[order[1]] if len(order) > 1 else 0
    C2 = _round8(max(second - C1, (counts[bot] + 1) // 2, NTILE))
    cores = []
    th, bh = (counts[top] + 1) // 2, (counts[bot] + 1) // 2
    cores.append([(top, 0, th), (bot, 0, bh)])
    cores.append([(top, th, counts[top]), (bot, bh, counts[bot])])
    for e in mids:
        cut = min(C1, counts[e])
        cores.append([(e, 0, cut), (e, cut, counts[e])])
    return C1, C2, cores


def kernel(x, gate_w, gate_b, w1, b1, w2, b2, _trace=False):
    x = np.ascontiguousarray(np.asarray(x, dtype=np.float32))
    gate_w = np.asarray(gate_w, dtype=np.float32)
    gate_b = np.asarray(gate_b, dtype=np.float32)
    w1 = np.asarray(w1, dtype=np.float32)
    b1 = np.asarray(b1, dtype=np.float32)
    w2 = np.asarray(w2, dtype=np.float32)
    b2 = np.asarray(b2, dtype=np.float32)

    B, S, D = x.shape
    E = gate_w.shape[1]
    H = w1.shape[2]
    assert E == N_CORES
    T = B * S
    x_flat = x.reshape(T, D)

    top_w, top_idx = _route(x_flat, gate_w, gate_b)

    toks, gvals = [], []
    for e in range(E):
        mask = top_idx == e  # [T, K]; at most one True per row
        t_ids = np.nonzero(mask.any(axis=1))[0]
        toks.append(t_ids)
        gvals.append(top_w[mask].astype(np.float32))
    counts = [len(t) for t in toks]
    C1, C2, cores = _assign_slots(counts)
    caps = (C1, C2)
    np_mm = _NP_MM_DT[MM_DT]

    in_maps = []
    for core in cores:
        xts, gs = [], []
        m = {}
        for s, (cap, (e, lo, hi)) in enumerate(zip(caps, core)):
            cnt = hi - lo
            XT = np.zeros((D, cap), np.float32)
            XT[:, :cnt] = x_flat[toks[e][lo:hi]].T
            G = np.zeros(cap, np.float32)
            G[:cnt] = gvals[e][lo:hi]
            xts.append(_pack_xt(XT, cap, D))
            gs.append(G)
            sn = "ab"[s]
            w1p, b1p, w2p, b2p = _pack_weights(w1[e], b1[e], w2[e], b2[e], D, H)
            m[f"w1{sn}"], m[f"b1{sn}"] = w1p, b1p
            m[f"w2{sn}"], m[f"b2{sn}"] = w2p, b2p
        m["xt"] = np.ascontiguousarray(np.concatenate(xts).astype(np_mm))
        m["g"] = np.ascontiguousarray(np.concatenate(gs).reshape(1, C1 + C2))
        in_maps.append(m)

    nc = _build_program(C1, C2, D, H, MM_DT)
    res = _run(nc, in_maps, trace=_trace)
    global _LAST_RES
    _LAST_RES = res

    out_flat = np.zeros((T, D), np.float32)
    tiles = _tile_sizes(C1) + _tile_sizes(C2)
    for i, core in enumerate(cores):
        outT = _unpack_out(res.results[i]["out"], tiles, D)
        for s, (cap, (e, lo, hi)) in enumerate(zip(caps, core)):
            cnt = hi - lo
            c0 = s * C1
            out_flat[toks[e][lo:hi]] += outT[:, c0 : c0 + cnt].T

    out = out_flat.reshape(B, S, D)
    if _trace:
        return out, res.exec_time_ns
    return out


# revision 19
# speedup vs baseline: 1.0857x; 1.0314x over previous
"""MoE (top-2 of 8 experts) Trainium2 kernel.

Strategy: expert-parallel across the 8 NeuronCores with 2-slot load
balancing. The router (a tiny [T,512]@[512,8] matmul + softmax + top-k,
~0.02% of the layer's FLOPs) runs on host bit-identically to the
reference (jax on CPU). Tokens are gathered per expert on host and
assigned to per-core (segment A, segment B) slots of fixed sizes
(C1, C2): the heaviest expert is split across two cores' A slots, the
lightest across those cores' B slots, and each remaining expert fills
one core's A+B. This balances per-core work to
max(2nd-heaviest, heaviest/2 + lightest/2) tokens instead of padding
every core to the heaviest expert.

Each segment computes its expert's FFN in a transposed layout (features
on partitions, tokens on the moving/free axis):

    outT = (w2.T @ gelu(w1.T @ xT + b1) + b2) * gate

so both matmuls chain on the TensorEngine with no transposes, and the
b1/b2 biases are free per-partition operands. The gate multiply uses a
partition-broadcast gate row. Host scatter-adds the two expert
contributions per token back into the full [B,S,D] output.

Matmul operands are bf16 (same 1 col/cycle PE rate as fp32r, half the
HBM traffic, FWL weight loads; ~4e-3 rel err end to end, vs the 2e-2
budget). PSUM accumulation stays fp32. ~30 dummy matmuls on a memset
scratch tile run while the first input DMAs stream, lifting the PE HAM
clock gate from 1.2 to 2.4 GHz before the real matmul stream begins.

All device inputs are packed on host into contiguous blocks laid out in
exactly the order the kernel consumes them: HWDGE drains the sync ring
FIFO, so consumption-ordered contiguous blocks give both full DMA
bandwidth and earliest possible compute start. Segment B's weights sit
behind segment A's in the ring and stream during segment A's compute.
"""

import os
import sys

sys.path.insert(0, "/opt/trn_rl_repo")

import ml_dtypes
import numpy as np

TOP_K = 2
N_CORES = 8
P = 128  # SBUF partitions

# Matmul dtype: "bfloat16" (1 cyc/row, FWL weight loads, half the DMA
# bytes, ~4e-3 rel err), "float32r" (1 cyc/row at N>=256, ~2e-4 rel err)
# or "float32" (exact, 4 cyc/row).
MM_DT = os.environ.get("MOE_MM_DT", "bfloat16")
NTILE = 512  # moving-operand (token) tile; PSUM-bank cap for f32 accum
ACT_FUNC = os.environ.get("MOE_ACT_FUNC", "Gelu")  # CoreSim lacks Gelu; Tanh for sim
N_WARM = 30  # dummy matmuls to lift the PE HAM clock-gate during input DMA

_NP_MM_DT = {
    "bfloat16": ml_dtypes.bfloat16,
    "float32r": np.float32,
    "float32": np.float32,
}


def _route(x_flat, gate_w, gate_b):
    """Reference router, bit-identical: jax on CPU."""
    import jax
    import jax.numpy as jnp

    with jax.default_device(jax.devices("cpu")[0]):
        logits = jnp.asarray(x_flat) @ jnp.asarray(gate_w) + jnp.asarray(gate_b)
        raw_weights = jax.nn.softmax(logits, axis=-1)
        top_w, top_idx = jax.lax.top_k(raw_weights, TOP_K)
        return np.asarray(top_w), np.asarray(top_idx)


def _tile_sizes(C):
    return [min(NTILE, C - c0) for c0 in range(0, C, NTILE)]


def _pack_xt(XT, C, D):
    """[D, C] -> consumption-ordered per-tile [P][KT][csz] blocks."""
    KT = D // P
    blocks = []
    for i, csz in enumerate(_tile_sizes(C)):
        c0 = i * NTILE
        blocks.append(
            XT.reshape(KT, P, C)[:, :, c0 : c0 + csz].transpose(1, 0, 2).ravel()
        )
    return np.concatenate(blocks)


def _pack_weights(w1e, b1e, w2e, b2e, D, H):
    """One expert's weights in the kernel's blocked layouts.

    w1: [4, P, 4, KT, P]  — four 4-m-block DMA groups, partition-major
    w2: [2, P, 8, D]      — two 8-m-block DMA groups, partition-major
    """
    np_mm = _NP_MM_DT[MM_DT]
    KT, MT, DT = D // P, H // P, D // P
    w1m = w1e.reshape(KT, P, MT, P).transpose(2, 1, 0, 3)  # [MT, P, KT, P]
    w2m = w2e.reshape(MT // 4, 4, P, D).transpose(0, 2, 1, 3)  # [MTG, P, 4, D]
    return (
        np.ascontiguousarray(
            w1m.reshape(4, 4, P, KT, P).transpose(0, 2, 1, 3, 4).astype(np_mm)
        ),
        np.ascontiguousarray(b1e.reshape(MT, P).T),
        np.ascontiguousarray(
            w2m.reshape(2, 2, P, 4, D)
            .transpose(0, 2, 1, 3, 4)
            .reshape(2, P, 8, D)
            .astype(np_mm)
        ),
        np.ascontiguousarray(b2e.reshape(DT, P).T),
    )


def _unpack_out(flat, tiles, D):
    """Per-tile [P][DT*csz] blocks -> outT [D, sum(tiles)]."""
    DT = D // P
    flat = np.asarray(flat).astype(np.float32)
    outT = np.empty((D, sum(tiles)), np.float32)
    off = 0
    c0 = 0
    for csz in tiles:
        blk = flat[off : off + P * DT * csz].reshape(P, DT, csz)
        outT[:, c0 : c0 + csz] = blk.transpose(1, 0, 2).reshape(D, csz)
        off += P * DT * csz
        c0 += csz
    return outT


def _build_program(C1, C2, D, H, mm_dt_name):
    """Build the per-core Bass program (identical on all cores).

    Two sequential segments (sizes C1 then C2), each with its own
    expert weight set; token/gate/output buffers are concatenated.
    """
    import concourse.bass as bass
    import concourse.mybir as mybir
    import concourse.tile as tile
    from concourse import bacc
    from concourse.tile_rust import add_dep_helper

    f32 = mybir.dt.float32
    mm_dt = getattr(mybir.dt, mm_dt_name)
    act = getattr(mybir.ActivationFunctionType, ACT_FUNC)
    KT = D // P  # 4  k-tiles for matmul1 (contraction over D)
    MT = H // P  # 16 m-tiles (H rows of hT)
    DT = D // P  # 4  d-tiles of the output
    MTG = MT // 4  # 4  w2 row-block groups
    C = C1 + C2

    nc = bacc.Bacc(None, target_bir_lowering=False, debug=False)
    xt_h = nc.dram_tensor("xt", [P * KT * C], mm_dt, kind="ExternalInput")
    g_h = nc.dram_tensor("g", [1, C], f32, kind="ExternalInput")
    w_h = {}
    for s in ("a", "b"):
        w_h[s] = (
            nc.dram_tensor(f"w1{s}", [4, P, 4, KT, P], mm_dt, kind="ExternalInput"),
            nc.dram_tensor(f"b1{s}", [P, MT], f32, kind="ExternalInput"),
            nc.dram_tensor(f"w2{s}", [2, P, 8, D], mm_dt, kind="ExternalInput"),
            nc.dram_tensor(f"b2{s}", [P, DT], f32, kind="ExternalInput"),
        )
    out_h = nc.dram_tensor("out", [P * DT * C], mm_dt, kind="ExternalOutput")

    with tile.TileContext(nc) as tc:
        with (
            tc.tile_pool(name="weights", bufs=1) as wpool,
            tc.tile_pool(name="xio", bufs=2) as xio,
            tc.tile_pool(name="gio", bufs=2) as gio,
            tc.tile_pool(name="oio", bufs=3) as oio,
            tc.tile_pool(name="hbuf", bufs=1) as hbuf,
            tc.tile_pool(name="ps1", bufs=4, space=bass.MemorySpace.PSUM) as ps1,
            # matmul2 keeps DT banks live across its whole m-loop; bufs=1
            # per d-tag (release happens at the DVE evacuation, early in
            # the next n-tile's matmul1 phase). 4 + 4 = 8 banks.
            tc.tile_pool(name="ps2", bufs=1, space=bass.MemorySpace.PSUM) as ps2,
        ):
            # DMA issue order == consumption order on the sync ring, and
            # DIRECT2D descriptor issue costs ~610ns each — the issue
            # RATE, not gating, dominates the upfront stream. So: few,
            # large DMAs, ordered by first consumption, all ungated
            # (everything upfront is segment-A data). Segment-B weights
            # are emitted inside the loop after tile A1's xt load, gated
            # on tile A1's first activation — a quiet window long before
            # segment B's compute begins.
            sizes_a, sizes_b = _tile_sizes(C1), _tile_sizes(C2)

            def load_weights(s, emitted):
                """w1 in 4 4-block groups; b1; w2 in 2 8-block groups."""
                w1h, b1h, w2h, b2h = w_h[s]
                w1_t, w2_t = [], []
                w1_t.append(wpool.tile([P, 4, KT, P], mm_dt, name=f"w1{s}_0"))
                emitted.append(nc.sync.dma_start(out=w1_t[0], in_=w1h.ap()[0]))
                b1_sb = wpool.tile([P, MT], f32, name=f"b1{s}")
                emitted.append(nc.sync.dma_start(out=b1_sb, in_=b1h.ap()))
                for gi in range(1, 4):
                    t = wpool.tile([P, 4, KT, P], mm_dt, name=f"w1{s}_{gi}")
                    emitted.append(nc.sync.dma_start(out=t, in_=w1h.ap()[gi]))
                    w1_t.append(t)
                b2_sb = wpool.tile([P, DT], f32, name=f"b2{s}")
                emitted.append(nc.sync.dma_start(out=b2_sb, in_=b2h.ap()))
                for gi in range(2):
                    t = wpool.tile([P, 8, D], mm_dt, name=f"w2{s}_{gi}")
                    emitted.append(nc.sync.dma_start(out=t, in_=w2h.ap()[gi]))
                    w2_t.append(t)
                return (w1_t, b1_sb, w2_t, b2_sb)

            xt0 = xio.tile([P, KT, sizes_a[0]], mm_dt, tag="xt", name="xt0")
            nc.sync.dma_start(
                out=xt0,
                in_=xt_h.ap()[0 : P * KT * sizes_a[0]].rearrange(
                    "(p kt c) -> p kt c", p=P, kt=KT
                ),
            )
            wts_a = load_weights("a", [])
            # broadcast the gate row across partitions in one HWDGE DMA;
            # last upfront: first consumer is tile A0's evacuation
            g_full = gio.tile([P, C], f32, name="g_full")
            nc.sync.dma_start(out=g_full, in_=g_h.ap().partition_broadcast(P))

            def emit_seg_b_weights():
                dmas = []
                wts = load_weights("b", dmas)
                return wts, dmas

            # PE warmup: the HAM clock gate holds the PE at 1.2 GHz until
            # ~3.4us of sustained matmul activity. Real matmuls can't
            # start until xt0+w1a_0 land (~9us in); dummy matmuls on a
            # memset scratch tile need no DMA, so they run from t~7.5us
            # and lift the clock toward 2.4 GHz before the real stream
            # begins. They borrow ps2_0's PSUM bank (released long before
            # matmul2 needs it).
            warm_sb = wpool.tile([P, P], mm_dt, name="warm")
            nc.gpsimd.memset(warm_sb[:, :], 0.0)
            warm_ps = ps2.tile([P, NTILE], f32, tag="ps2_0", name="ps2_0")
            for _ in range(N_WARM):
                nc.tensor.matmul(
                    warm_ps[:, 0:P], lhsT=warm_sb, rhs=warm_sb, start=True, stop=True
                )

            state = {
                "xt_off": 0,
                "out_off": 0,
                "c0": 0,
                "prev_first_act": None,
                "wts_b": None,
            }

            def run_segment(seg, sizes, wts, xt_first=None):
                w1_t, b1_sb, w2_t, b2_sb = wts
                for n, csz in enumerate(sizes):
                    if n == 0 and xt_first is not None:
                        xt_t = xt_first
                    else:
                        xt_t = xio.tile([P, KT, csz], mm_dt, tag="xt", name="xt")
                        dma = nc.sync.dma_start(
                            out=xt_t,
                            in_=xt_h.ap()[
                                state["xt_off"] : state["xt_off"] + P * KT * csz
                            ].rearrange("(p kt c) -> p kt c", p=P, kt=KT),
                        )
                        if state["prev_first_act"] is not None:
                            add_dep_helper(
                                dma.ins,
                                state["prev_first_act"].ins,
                                reason="stagger xt load",
                            )
                    state["xt_off"] += P * KT * csz
                    # segment-B weights enter the ring right after tile
                    # A1's xt load; their gate (act A1 m0) is attached
                    # below once that activation exists
                    pending_b = None
                    if seg == "a" and n == min(1, len(sizes) - 1):
                        state["wts_b"], pending_b = emit_seg_b_weights()
                    g_t = g_full[:, state["c0"] : state["c0"] + csz]
                    state["c0"] += csz
                    hT = hbuf.tile([P, MT, csz], mm_dt, tag="hT", name="hT")
                    first_act = None
                    for m in range(MT):
                        pst = ps1.tile([P, csz], f32, tag="ps1", name="ps1")
                        for kt in range(KT):
                            nc.tensor.matmul(
                                pst,
                                lhsT=w1_t[m // 4][:, m % 4, kt, :],
                                rhs=xt_t[:, kt, :],
                                start=(kt == 0),
                                stop=(kt == KT - 1),
                            )
                        a = nc.scalar.activation(
                            out=hT[:, m, :],
                            in_=pst,
                            func=act,
                            bias=b1_sb[:, m : m + 1],
                            scale=1.0,
                        )
                        if m == 0:
                            first_act = a
                            if pending_b is not None:
                                for dma in pending_b:
                                    add_dep_helper(
                                        dma.ins, a.ins, reason="stagger segB weights"
                                    )
                    state["prev_first_act"] = first_act
                    # matmul2 with m as the OUTER loop: w2 blocks are
                    # consumed in DMA-arrival order. Needs DT live PSUM
                    # banks.
                    pso = [
                        ps2.tile([P, csz], f32, tag=f"ps2_{d}", name=f"ps2_{d}")
                        for d in range(DT)
                    ]
                    for m in range(MT):
                        for d in range(DT):
                            nc.tensor.matmul(
                                pso[d],
                                lhsT=w2_t[m // 8][:, m % 8, d * P : (d + 1) * P],
                                rhs=hT[:, m, :],
                                start=(m == 0),
                                stop=(m == MT - 1),
                            )
                    ot = oio.tile([P, DT, csz], mm_dt, tag="ot", name="ot")
                    for d in range(DT):
                        nc.vector.scalar_tensor_tensor(
                            out=ot[:, d, :],
                            in0=pso[d],
                            scalar=b2_sb[:, d : d + 1],
                            in1=g_t,
                            op0=mybir.AluOpType.add,
                            op1=mybir.AluOpType.mult,
                        )
                    # one store per tile; [P][DT*csz] keeps the DMA
                    # descriptor 2D-contiguous (4KB partition lines)
                    nc.sync.dma_start(
                        out=out_h.ap()[
                            state["out_off"] : state["out_off"] + P * DT * csz
                        ].rearrange("(p x) -> p x", p=P),
                        in_=ot[:, :, :],
                    )
                    state["out_off"] += P * DT * csz

            run_segment("a", sizes_a, wts_a, xt_first=xt0)
            run_segment("b", sizes_b, state["wts_b"])

    nc.compile()
    return nc


def _run(nc, in_maps, trace=False):
    from concourse.bass_utils import run_bass_kernel_spmd

    if trace:
        # register the NTFF profiling hook (missing antenv.axon_hooks shim)
        import types

        import antenv

        if not hasattr(antenv, "axon_hooks"):
            mod = types.ModuleType("antenv.axon_hooks")
            _hook = [None]
            mod.set_axon_ntff_profile_hook = lambda h: _hook.__setitem__(0, h)
            mod.get_axon_ntff_profile_hook = lambda: _hook[0]
            sys.modules["antenv.axon_hooks"] = mod
            antenv.axon_hooks = mod
            from trn_agent_boot.trn_boot import _ntff_profile_via_ctypes

            mod.set_axon_ntff_profile_hook(
                _ntff_profile_via_ctypes("/opt/axon/libaxon_pjrt.so")
            )
    return run_bass_kernel_spmd(
        nc, in_maps, core_ids=list(range(N_CORES)), trace=trace
    )


def _round8(v):
    return ((int(v) + 7) // 8) * 8


def _assign_slots(counts):
    """2-slot balanced assignment.

    Returns (C1, C2, cores) where cores[i] = [(expert, lo, hi), ...] of
    length 2: core i computes tokens [lo:hi) of each expert in its A/B
    slots. The heaviest expert splits across cores 0,1's A slots; the
    lightest across their B slots; each middle expert fills one core.
    """
    order = sorted(range(len(counts)), key=lambda e: -counts[e])
    top, bot, mids = order[0], order[-1], order[1:-1]
    C1 = _round8(max((counts[top] + 1) // 2, NTILE))
    second = counts[order[1]] if len(order) > 1 else 0
    C2 = _round8(max(second - C1, (counts[bot] + 1) // 2, NTILE))
    cores = []
    th, bh = (counts[top] + 1) // 2, (counts[bot] + 1) // 2
    cores.append([(top, 0, th), (bot, 0, bh)])
    cores.append([(top, th, counts[top]), (bot, bh, counts[bot])])
    for e in mids:
        cut = min(C1, counts[e])
        cores.append([(e, 0, cut), (e, cut, counts[e])])
    return C1, C2, cores


def kernel(x, gate_w, gate_b, w1, b1, w2, b2, _trace=False):
    x = np.ascontiguousarray(np.asarray(x, dtype=np.float32))
    gate_w = np.asarray(gate_w, dtype=np.float32)
    gate_b = np.asarray(gate_b, dtype=np.float32)
    w1 = np.asarray(w1, dtype=np.float32)
    b1 = np.asarray(b1, dtype=np.float32)
    w2 = np.asarray(w2, dtype=np.float32)
    b2 = np.asarray(b2, dtype=np.float32)

    B, S, D = x.shape
    E = gate_w.shape[1]
    H = w1.shape[2]
    assert E == N_CORES
    T = B * S
    x_flat = x.reshape(T, D)

    top_w, top_idx = _route(x_flat, gate_w, gate_b)

    toks, gvals = [], []
    for e in range(E):
        mask = top_idx == e  # [T, K]; at most one True per row
        t_ids = np.nonzero(mask.any(axis=1))[0]
        toks.append(t_ids)
        gvals.append(top_w[mask].astype(np.float32))
    counts = [len(t) for t in toks]
    C1, C2, cores = _assign_slots(counts)
    caps = (C1, C2)
    np_mm = _NP_MM_DT[MM_DT]

    in_maps = []
    for core in cores:
        xts, gs = [], []
        m = {}
        for s, (cap, (e, lo, hi)) in enumerate(zip(caps, core)):
            cnt = hi - lo
            XT = np.zeros((D, cap), np.float32)
            XT[:, :cnt] = x_flat[toks[e][lo:hi]].T
            G = np.zeros(cap, np.float32)
            G[:cnt] = gvals[e][lo:hi]
            xts.append(_pack_xt(XT, cap, D))
            gs.append(G)
            sn = "ab"[s]
            w1p, b1p, w2p, b2p = _pack_weights(w1[e], b1[e], w2[e], b2[e], D, H)
            m[f"w1{sn}"], m[f"b1{sn}"] = w1p, b1p
            m[f"w2{sn}"], m[f"b2{sn}"] = w2p, b2p
        m["xt"] = np.ascontiguousarray(np.concatenate(xts).astype(np_mm))
        m["g"] = np.ascontiguousarray(np.concatenate(gs).reshape(1, C1 + C2))
        in_maps.append(m)

    nc = _build_program(C1, C2, D, H, MM_DT)
    res = _run(nc, in_maps, trace=_trace)
    global _LAST_RES
    _LAST_RES = res

    out_flat = np.zeros((T, D), np.float32)
    tiles = _tile_sizes(C1) + _tile_sizes(C2)
    for i, core in enumerate(cores):
        outT = _unpack_out(res.results[i]["out"], tiles, D)
        for s, (cap, (e, lo, hi)) in enumerate(zip(caps, core)):
            cnt = hi - lo
            c0 = s * C1
            out_flat[toks[e][lo:hi]] += outT[:, c0 : c0 + cnt].T

    out = out_flat.reshape(B, S, D)
    if _trace:
        return out, res.exec_time_ns
    return out
